# revision 17
# baseline (speedup 1.0000x reference)
"""GATv2 (3-layer, 4-head) on 8 Trainium2 NeuronCores.

Strategy (graph/data parallel, per sharding hint):
- Nodes partitioned across 8 cores by dst (6250 real + 22 pad -> 6272/core).
- Host sorts edges by dst, packs segments (consecutive dsts) into fixed
  supertiles of 49 dsts x <=1024 edge slots. Segment softmax + scatter-add
  become per-supertile matmuls against 0/1 segment matrices; outputs land in
  transposed layout h_T[feat, node] which feeds the next layer's matmuls
  directly. AllGather replicates h between layers.
- Layer 1 needs no gather: host pre-expands x[src] per edge slot (input
  rearrangement only), so Gl comes from a matmul. Layers 2/3 gather xl rows
  via indirect DMA (128 rows/instruction).
"""
import math
import numpy as np

import concourse.bass as bass
import concourse.bacc as bacc
import concourse.mybir as mybir
import concourse.tile as tile
from concourse.bass_utils import run_bass_kernel_spmd

F32 = mybir.dt.float32
F16 = mybir.dt.float16
I32 = mybir.dt.int32
AX = mybir.AxisListType
ALU = mybir.AluOpType
ACTF = mybir.ActivationFunctionType

# ---------------- problem geometry (hardcoded per spec) ----------------
N, E = 50000, 800000
DIM_IN, DIM_H, DIM_OUT, HEADS = 128, 32, 16, 4
HID = DIM_H * HEADS      # 128
FINAL = DIM_OUT * HEADS  # 64
NEG_SLOPE = 0.2

NCORE = 8
NLOC = N // NCORE        # 6250 real nodes per core
SEG = 49                 # dst segments per supertile
NT = 128                 # supertiles per core;  SEG*NT = 6272 = NP
NP = SEG * NT            # padded nodes per core (49*128 = 6272)
NTAB = NCORE * NP        # padded global node count (50176)
ET = 1024                # edge slots per supertile
NB = ET // 128           # gather blocks per supertile (8)
PAD_SRC = 1 << 30        # oob src id for pad slots (gather skipped)

USE_ACT_LRELU = True     # leaky-relu on scalar engine (not in CoreSim)
DEBUG_T0 = False         # emit intermediates of supertile 0, layer 1
TRACE = False            # set by test harness to capture NTFF exec time
LAST_EXEC_NS = None
FOR_UNROLL = 8


# ---------------- host-side preprocessing ----------------
def _preprocess(x, edge_index):
    """Returns per-core host arrays + shared consts."""
    src = np.asarray(edge_index[0], dtype=np.int64)
    dst = np.asarray(edge_index[1], dtype=np.int64)
    loops = np.arange(N, dtype=np.int64)
    src = np.concatenate([src, loops])
    dst = np.concatenate([dst, loops])

    core = dst // NLOC
    src_p = (src // NLOC) * NP + (src % NLOC)   # padded global ids
    dst_l = dst % NLOC                           # local id on its core

    x = np.asarray(x, dtype=np.float32)

    per_core = []
    for c in range(NCORE):
        m = core == c
        s_c = src_p[m]
        d_c = dst_l[m]
        order = np.argsort(d_c, kind="stable")
        s_c = s_c[order]
        d_c = d_c[order]
        # counts per local dst (every real dst has >=1 edge via self loop)
        cnt = np.bincount(d_c, minlength=NP)
        # fake edges for pad dsts so no segment is empty
        n_fake = NP - NLOC
        s_c = np.concatenate([s_c, np.full(n_fake, c * NP, dtype=np.int64)])
        d_c = np.concatenate([d_c, np.arange(NLOC, NP, dtype=np.int64)])
        cnt[NLOC:] = 1
        # supertile t covers dsts [t*SEG, (t+1)*SEG)
        tile_of_dst = np.arange(NP) // SEG
        tile_edges = np.bincount(tile_of_dst, weights=cnt, minlength=NT).astype(np.int64)
        if tile_edges.max() > ET:
            raise RuntimeError(f"supertile overflow: {tile_edges.max()} > {ET}")
        # slot assignment: edges sorted by dst; segment of edge = dst % SEG
        starts = np.zeros(NP + 1, dtype=np.int64)
        starts[1:] = np.cumsum(cnt)
        src_slot = np.full((NT, ET), PAD_SRC, dtype=np.int64)
        seg_slot = np.full((NT, ET), SEG, dtype=np.float32)  # pad -> trash seg
        # edges of tile t occupy the contiguous range in the dst-sorted order
        edge_tile = tile_of_dst[d_c]
        pos_in_tile = np.zeros(len(d_c), dtype=np.int64)
        t_starts = np.zeros(NT + 1, dtype=np.int64)
        t_starts[1:] = np.cumsum(tile_edges)
        pos_in_tile = np.arange(len(d_c)) - t_starts[edge_tile]
        src_slot[edge_tile, pos_in_tile] = s_c
        seg_slot[edge_tile, pos_in_tile] = (d_c % SEG).astype(np.float32)

        # device layouts -------------------------------------------------
        # srcseg int32 [NT*128, 2*NB]: [p, j<NB] = src of slot j*128+p;
        #                              [p, NB+j] = f32 bits of seg id
        srcseg = np.zeros((NT * 128, 2 * NB), dtype=np.int32)
        src_r = src_slot.reshape(NT, NB, 128)          # [t, j, p]
        seg_r = seg_slot.reshape(NT, NB, 128)
        for t in range(NT):
            srcseg[t * 128:(t + 1) * 128, :NB] = src_r[t].T.astype(np.int32)
            srcseg[t * 128:(t + 1) * 128, NB:] = seg_r[t].T.astype(np.float32).view(np.int32)
        # xeT f16 [128, NT*ET]: column (t*ET + q) = x[src of slot q] (0 for pads)
        xe = np.zeros((NT * ET, DIM_IN), dtype=np.float16)
        valid = src_slot.reshape(-1) != PAD_SRC
        gsrc = src_slot.reshape(-1)[valid]
        # padded id -> original id
        orig = (gsrc // NP) * NLOC + (gsrc % NP)
        xe[valid] = x[orig].astype(np.float16)
        xeT = np.ascontiguousarray(xe.T)               # [128, NT*ET]
        xT_loc = np.zeros((128, NP), dtype=np.float16)
        xT_loc[:, :NLOC] = x[c * NLOC:(c + 1) * NLOC].T.astype(np.float16)
        per_core.append({"srcseg": srcseg, "xeT": xeT, "xT_loc": xT_loc})
    return per_core


def _consts(Wl1, Wr1, att1, b1, Wl2, Wr2, att2, b2, Wl3, Wr3, att3, b3):
    att1f = np.asarray(att1, np.float32).reshape(-1)   # [128]
    att2f = np.asarray(att2, np.float32).reshape(-1)
    att3f = np.asarray(att3, np.float32).reshape(-1)   # [64]
    e4 = np.zeros((HEADS, HID), np.float32)
    for h in range(HEADS):
        e4[h, h * DIM_H:(h + 1) * DIM_H] = 1.0
    e43 = np.zeros((HEADS, FINAL), np.float32)
    for h in range(HEADS):
        e43[h, h * DIM_OUT:(h + 1) * DIM_OUT] = 1.0
    c = {
        "wl1": np.asarray(Wl1, np.float16), "wr1": np.asarray(Wr1, np.float16),
        "wl2": np.asarray(Wl2, np.float16), "wr2": np.asarray(Wr2, np.float16),
        "wl3": np.asarray(Wl3, np.float16), "wr3": np.asarray(Wr3, np.float16),
        "attr1": np.tile(att1f, (128, 1)),   # [128,128] f32
        "attr2": np.tile(att2f, (128, 1)),
        "attr3": np.tile(att3f, (128, 1)),   # [128,64]
        "b1c": np.asarray(b1, np.float32).reshape(HID, 1),
        "b2c": np.asarray(b2, np.float32).reshape(HID, 1),
        "b3r": np.tile(np.asarray(b3, np.float32).reshape(1, FINAL), (128, 1)),
        "iota": np.tile(np.arange(128, dtype=np.float32)[None, :], (128, 1)),
        "ident": np.eye(128, dtype=np.float16),
        "e4": e4, "e43": e43,
    }
    return c


# ---------------- device kernel ----------------
def _build():
    nc = bacc.Bacc("TRN2", target_bir_lowering=False, debug=False,
                   enable_asserts=False, num_devices=NCORE)

    def din(name, shape, dt):
        return nc.dram_tensor(name, shape, dt, kind="ExternalInput").ap()

    srcseg = din("srcseg", [NT * 128, 2 * NB], I32)
    xeT = din("xeT", [128, NT * ET], F16)
    xT_loc = din("xT_loc", [128, NP], F16)
    wl1 = din("wl1", [128, HID], F16); wr1 = din("wr1", [128, HID], F16)
    wl2 = din("wl2", [128, HID], F16); wr2 = din("wr2", [128, HID], F16)
    wl3 = din("wl3", [128, FINAL], F16); wr3 = din("wr3", [128, FINAL], F16)
    attr1 = din("attr1", [128, HID], F32)
    attr2 = din("attr2", [128, HID], F32)
    attr3 = din("attr3", [128, FINAL], F32)
    b1c = din("b1c", [HID, 1], F32)
    b2c = din("b2c", [HID, 1], F32)
    b3r = din("b3r", [128, FINAL], F32)
    iota = din("iota", [128, 128], F32)
    ident = din("ident", [128, 128], F16)
    e4 = din("e4", [HEADS, HID], F32)
    e43 = din("e43", [HEADS, FINAL], F32)

    outp = nc.dram_tensor("outp", [NP, FINAL], F32, kind="ExternalOutput").ap()
    if DEBUG_T0:
        dbg_gl = nc.dram_tensor("dbg_gl", [128, NB * HID], F32, kind="ExternalOutput").ap()
        dbg_S = nc.dram_tensor("dbg_S", [128, NB * 128], F32, kind="ExternalOutput").ap()
        dbg_z = nc.dram_tensor("dbg_z", [HID, SEG + 1], F32, kind="ExternalOutput").ap()
        dbg_ev = nc.dram_tensor("dbg_ev", [128, NB * HEADS], F32, kind="ExternalOutput").ap()
        dbg_h = nc.dram_tensor("dbg_h", [128, SEG], F32, kind="ExternalOutput").ap()

    with tile.TileContext(nc) as tc:
        with tc.tile_pool(name="cst", bufs=1) as cst, \
             tc.tile_pool(name="hbuf", bufs=1) as hbuf, \
             tc.tile_pool(name="sb", bufs=2) as sb, \
             tc.tile_pool(name="ps", bufs=1, space="PSUM") as ps, \
             tc.tile_pool(name="ps2", bufs=2, space="PSUM") as ps2, \
             tc.tile_pool(name="dram", bufs=1, space="DRAM") as dram:

            # ---- persistent constants in SBUF ----
            def load_const(apx, dt, shape=None, tag=None):
                shape = shape or list(apx.shape)
                t = cst.tile(shape, dt, tag=tag or apx.tensor.name)
                nc.sync.dma_start(out=t[:], in_=apx[:])
                return t

            wl1_s = load_const(wl1, F16); wr1_s = load_const(wr1, F16)
            wl2_s = load_const(wl2, F16); wr2_s = load_const(wr2, F16)
            wl3_s = load_const(wl3, F16); wr3_s = load_const(wr3, F16)
            attr1_s = load_const(attr1, F32); attr2_s = load_const(attr2, F32)
            attr3_s = load_const(attr3, F32)
            b1c_s = load_const(b1c, F32); b2c_s = load_const(b2c, F32)
            b3r_s = load_const(b3r, F32)
            iota_s = load_const(iota, F32)
            ident_s = load_const(ident, F16)
            ident32_s = cst.tile([128, 128], F32, tag="ident32")
            nc.vector.tensor_copy(ident32_s[:], ident_s[:])
            e4_s = load_const(e4, F32); e43_s = load_const(e43, F32)
            zeros_s = cst.tile([128, SEG + 1], F32, tag="zeros")
            nc.vector.memset(zeros_s[:], 0.0)

            # ---- persistent hT (reused across layers) ----
            hT = hbuf.tile([128, NP], F16, tag="hT")

            # ---- internal DRAM ----
            xl2 = dram.tile([NTAB, HID], F32, tag="xl2")
            xl3 = dram.tile([NTAB, FINAL], F32, tag="xl3")
            xr1 = dram.tile([NP + 128, HID], F16, tag="xr1")
            xr2 = dram.tile([NP + 128, HID], F16, tag="xr2")
            xr3 = dram.tile([NP + 128, FINAL], F16, tag="xr3")
            cc_in1 = dram.tile([128 * NP], F16, tag="cc_in1")
            cc_out1 = dram.tile([NCORE * 128, NP], F16, tag="cc_out1")
            cc_in2 = dram.tile([128 * NP], F16, tag="cc_in2")
            cc_out2 = dram.tile([NCORE * 128, NP], F16, tag="cc_out2")
            opreT = dram.tile([FINAL, NP], F32, tag="opreT")

            # zero the xr over-read tails
            ztail = sb.tile([128, HID], F16, tag="ztail")
            nc.vector.memset(ztail[:], 0.0)
            nc.sync.dma_start(out=xr1[NP:NP + 128, :], in_=ztail[:, :HID])
            nc.sync.dma_start(out=xr2[NP:NP + 128, :], in_=ztail[:, :HID])
            nc.sync.dma_start(out=xr3[NP:NP + 128, :], in_=ztail[:, :FINAL])

            # ================= edge phase =================
            dbg_cell = {}

            def edge_tile(i, layer):
                """One supertile: 1024 edge slots, SEG dst segments."""
                F = HID if layer < 3 else FINAL
                attr_s = (attr1_s, attr2_s, attr3_s)[layer - 1]
                ss = sb.tile([128, 2 * NB], I32, tag="ss")
                nc.sync.dma_start(out=ss[:], in_=srcseg[bass.ds(i * 128, 128), :])
                segf = ss[:].bitcast(F32)[:, NB:2 * NB]        # [128, NB] f32

                # --- Gl [128e, NB*F] f32 ---
                GRP = 512 // F                 # blocks per PSUM bank
                ngrp = (NB + GRP - 1) // GRP
                gl = sb.tile([128, NB * F], F32, tag="gl")
                if layer == 1:
                    xt = sb.tile([128, ET], F16, tag="xt")
                    nc.sync.dma_start(out=xt[:], in_=xeT[:, bass.ds(i * ET, ET)])
                    for g in range(ngrp):
                        nblk = min(GRP, NB - g * GRP)
                        gp = ps2.tile([128, 512], F32, tag="pbig", space="PSUM")
                        for jj in range(nblk):
                            j = g * GRP + jj
                            nc.tensor.matmul(
                                gp[:, jj * F:(jj + 1) * F],
                                lhsT=xt[:, j * 128:(j + 1) * 128],
                                rhs=wl1_s[:], start=True, stop=True)
                        nc.scalar.copy(gl[:, g * GRP * F:(g * GRP + nblk) * F],
                                       gp[:, :nblk * F])
                else:
                    tbl = xl2 if layer == 2 else xl3
                    for j in range(NB):
                        nc.gpsimd.indirect_dma_start(
                            out=gl[:, j * F:(j + 1) * F], out_offset=None,
                            in_=tbl[:],
                            in_offset=bass.IndirectOffsetOnAxis(ap=ss[:, j:j + 1], axis=0),
                            bounds_check=NTAB - 1, oob_is_err=False)

                # --- segment matrix S [128e, NB, SEG+1] f16 ---
                S = sb.tile([128, NB, 128], F16, tag="S")
                nc.vector.tensor_tensor(
                    out=S[:],
                    in0=segf.rearrange("p (b one) -> p b one", one=1).to_broadcast([128, NB, 128]),
                    in1=iota_s[:].rearrange("p (one s) -> p one s", one=1).to_broadcast([128, NB, 128]),
                    op=ALU.is_equal)

                # --- S_T via PE transpose ---
                st = sb.tile([128, NB * 128], F16, tag="st")
                nsgrp = (NB + 3) // 4
                for g in range(nsgrp):
                    nblk = min(4, NB - g * 4)
                    sp = ps2.tile([128, 512], F16, tag="psp", space="PSUM")
                    for jj in range(nblk):
                        j = g * 4 + jj
                        nc.tensor.transpose(sp[:, jj * 128:(jj + 1) * 128],
                                            S[:, j, :], ident_s[:])
                    nc.scalar.copy(st[:, g * 512:g * 512 + nblk * 128],
                                   sp[:, :nblk * 128])

                # --- xr_u rows for this tile's dsts ---
                xrt = (xr1, xr2, xr3)[layer - 1]
                xu = sb.tile([128, F], F16, tag="xu")
                nc.sync.dma_start(out=xu[:], in_=xrt[bass.ds(i * SEG, 128), :])

                # --- m = Gl + S_T.T @ xr_u ; leaky-relu ---
                lr = sb.tile([128, NB * F], F32, tag="lr")
                for g in range(ngrp):
                    nblk = min(GRP, NB - g * GRP)
                    mp = ps2.tile([128, 512], F32, tag="pbig", space="PSUM")
                    for jj in range(nblk):
                        j = g * GRP + jj
                        nc.tensor.matmul(mp[:, jj * F:(jj + 1) * F],
                                         lhsT=st[:, j * 128:(j + 1) * 128],
                                         rhs=xu[:], start=True, stop=True)
                    gsl = slice(g * GRP * F, (g * GRP + nblk) * F)
                    msb = sb.tile([128, 512], F32, tag="msb")
                    nc.vector.tensor_add(msb[:, :nblk * F], gl[:, gsl],
                                         mp[:, :nblk * F])
                    if USE_ACT_LRELU:
                        nc.scalar.activation(lr[:, gsl], msb[:, :nblk * F],
                                             ACTF.Lrelu, alpha=NEG_SLOPE)
                    else:
                        nc.vector.scalar_tensor_tensor(
                            out=lr[:, gsl], in0=msb[:, :nblk * F], scalar=NEG_SLOPE,
                            in1=msb[:, :nblk * F], op0=ALU.mult, op1=ALU.max)

                # --- e = per-head dot with att ---
                C = F // HEADS
                tm = sb.tile([128, NB * F], F32, tag="tm")
                nc.vector.tensor_tensor(
                    out=tm[:].rearrange("p (b h c) -> p b h c", b=NB, h=HEADS),
                    in0=lr[:].rearrange("p (b h c) -> p b h c", b=NB, h=HEADS),
                    in1=attr_s[:].rearrange("p (one h c) -> p one h c", one=1, h=HEADS)
                        .to_broadcast([128, NB, HEADS, C]),
                    op=ALU.mult)
                ev = sb.tile([128, NB * HEADS], F32, tag="ev")
                nc.vector.reduce_sum(
                    ev[:].rearrange("p (b h one) -> p b h one", b=NB, one=1),
                    tm[:].rearrange("p (b h c) -> p b h c", b=NB, h=HEADS),
                    axis=AX.X)
                exs = sb.tile([128, NB * HEADS], F16, tag="exs")
                nc.scalar.activation(exs[:], ev[:], ACTF.Exp)

                # --- P = Gl * ex (head-broadcast), cast f16 ---
                pp = sb.tile([128, NB * F], F16, tag="pp")
                nc.vector.tensor_tensor(
                    out=pp[:].rearrange("p (b h c) -> p b h c", b=NB, h=HEADS),
                    in0=gl[:].rearrange("p (b h c) -> p b h c", b=NB, h=HEADS),
                    in1=exs[:].rearrange("p (b h one) -> p b h one", b=NB, one=1)
                        .to_broadcast([128, NB, HEADS, C]),
                    op=ALU.mult)

                # --- segment sums: out_T [F, SEG+1], denom [H, SEG+1] ---
                otp = ps.tile([F, SEG + 1], F32, tag="otp", space="PSUM")
                dnp = ps.tile([HEADS, SEG + 1], F32, tag="dnp", space="PSUM")
                for j in range(NB):
                    nc.tensor.matmul(otp[:], lhsT=pp[:, j * F:(j + 1) * F],
                                     rhs=S[:, j, :SEG + 1],
                                     start=(j == 0), stop=(j == NB - 1))
                for j in range(NB):
                    nc.tensor.matmul(dnp[:], lhsT=exs[:, j * HEADS:(j + 1) * HEADS],
                                     rhs=S[:, j, :SEG + 1],
                                     start=(j == 0), stop=(j == NB - 1))
                rd = sb.tile([HEADS, SEG + 1], F32, tag="rd")
                nc.vector.reciprocal(rd[:], dnp[:])
                dxp = ps.tile([F, SEG + 1], F32, tag="dxp", space="PSUM")
                e4c = e4_s if layer < 3 else e43_s
                nc.tensor.matmul(dxp[:], lhsT=e4c[:], rhs=rd[:], start=True, stop=True)
                dxs = sb.tile([F, SEG + 1], F32, tag="dxs")
                nc.scalar.copy(dxs[:], dxp[:])
                z = sb.tile([F, SEG + 1], F32, tag="z")
                nc.vector.tensor_tensor(out=z[:], in0=otp[:], in1=dxs[:], op=ALU.mult)
                if DEBUG_T0 and isinstance(i, int) and i == 0 and layer == 1 and not dbg_cell:
                    dbg_cell.update(gl=gl, S=S, z=z, ev=ev)

                if layer < 3:
                    bc = b1c_s if layer == 1 else b2c_s
                    # elu(z + b) into hT[:, i*SEG : i*SEG+SEG]
                    t1 = sb.tile([F, SEG], F32, tag="t1")
                    nc.vector.scalar_tensor_tensor(
                        out=t1[:], in0=z[:, :SEG], scalar=bc[:, :1],
                        in1=zeros_s[:F, :SEG], op0=ALU.add, op1=ALU.min)
                    t2 = sb.tile([F, SEG], F32, tag="t2")
                    nc.scalar.activation(t2[:], t1[:], ACTF.Exp)
                    t3 = sb.tile([F, SEG], F32, tag="t3")
                    nc.vector.scalar_tensor_tensor(
                        out=t3[:], in0=z[:, :SEG], scalar=bc[:, :1],
                        in1=zeros_s[:F, :SEG], op0=ALU.add, op1=ALU.max)
                    nc.vector.scalar_tensor_tensor(
                        out=hT[:, bass.ds(i * SEG, SEG)], in0=t2[:], scalar=-1.0,
                        in1=t3[:], op0=ALU.add, op1=ALU.add)
                else:
                    nc.sync.dma_start(out=opreT[:, bass.ds(i * SEG, SEG)],
                                      in_=z[:, :SEG])

            # ================= table phase =================
            def xr_phase(wr_s, xrt, F):
                for c in range(NP // 128):
                    p = ps2.tile([128, F], F32, tag="pbig", space="PSUM")
                    nc.tensor.matmul(p[:], lhsT=hT[:, c * 128:(c + 1) * 128],
                                     rhs=wr_s[:], start=True, stop=True)
                    s = sb.tile([128, F], F16, tag="xrs")
                    nc.scalar.copy(s[:], p[:])
                    nc.sync.dma_start(out=xrt[c * 128:(c + 1) * 128, :], in_=s[:])

            def xl_phase(cc_out, wl_s, xlt, F):
                def body(ci):
                    for r in range(NCORE):
                        htc = sb.tile([128, 128], F16, tag="htc")
                        nc.sync.dma_start(
                            out=htc[:],
                            in_=cc_out[r * 128:(r + 1) * 128, bass.ds(ci * 128, 128)])
                        p = ps2.tile([128, F], F32, tag="pbig", space="PSUM")
                        nc.tensor.matmul(p[:], lhsT=htc[:], rhs=wl_s[:],
                                         start=True, stop=True)
                        s = sb.tile([128, F], F32, tag="xls")
                        nc.scalar.copy(s[:], p[:])
                        nc.sync.dma_start(
                            out=xlt[bass.ds(r * NP + ci * 128, 128), :], in_=s[:])
                tc.For_i_unrolled(0, NP // 128, 1, body, max_unroll=8)

            # ================= layer 1 =================
            nc.sync.dma_start(out=hT[:], in_=xT_loc[:])
            xr_phase(wr1_s, xr1, HID)

            if DEBUG_T0:
                edge_tile(0, 1)
                nc.sync.dma_start(out=dbg_gl[:], in_=dbg_cell["gl"][:])
                dbg_S32 = sb.tile([128, NB * 128], F32, tag="dbgS32")
                nc.vector.tensor_copy(dbg_S32[:], dbg_cell["S"][:].rearrange("p b s -> p (b s)"))
                nc.sync.dma_start(out=dbg_S[:], in_=dbg_S32[:])
                nc.sync.dma_start(out=dbg_z[:], in_=dbg_cell["z"][:])
                nc.sync.dma_start(out=dbg_ev[:], in_=dbg_cell["ev"][:])
                dbg_h32 = sb.tile([128, SEG], F32, tag="dbgh32")
                nc.vector.tensor_copy(dbg_h32[:], hT[:, :SEG])
                nc.sync.dma_start(out=dbg_h[:], in_=dbg_h32[:])

            def l1_body(i):
                edge_tile(i, 1)
            tc.For_i_unrolled(1 if DEBUG_T0 else 0, NT, 1, l1_body, max_unroll=FOR_UNROLL)

            nc.sync.dma_start(out=cc_in1[:].rearrange("(p n) -> p n", p=128), in_=hT[:])
            nc.gpsimd.collective_compute(
                "AllGather", ALU.bypass,
                ins=[cc_in1.opt()], outs=[cc_out1.opt()],
                replica_groups=[list(range(NCORE))])

            xr_phase(wr2_s, xr2, HID)
            xl_phase(cc_out1, wl2_s, xl2, HID)

            # ================= layer 2 =================
            def l2_body(i):
                edge_tile(i, 2)
            tc.For_i_unrolled(0, NT, 1, l2_body, max_unroll=FOR_UNROLL)

            nc.sync.dma_start(out=cc_in2[:].rearrange("(p n) -> p n", p=128), in_=hT[:])
            nc.gpsimd.collective_compute(
                "AllGather", ALU.bypass,
                ins=[cc_in2.opt()], outs=[cc_out2.opt()],
                replica_groups=[list(range(NCORE))])

            xr_phase(wr3_s, xr3, FINAL)
            xl_phase(cc_out2, wl3_s, xl3, FINAL)

            # ================= layer 3 =================
            def l3_body(i):
                edge_tile(i, 3)
            tc.For_i_unrolled(0, NT, 1, l3_body, max_unroll=FOR_UNROLL)

            # ================= final: bias + log_softmax =================
            for c in range(NP // 128):
                ot = sb.tile([FINAL, 128], F32, tag="ot")
                nc.sync.dma_start(out=ot[:], in_=opreT[:, c * 128:(c + 1) * 128])
                tp = ps2.tile([128, FINAL], F32, tag="pbig", space="PSUM")
                nc.tensor.transpose(tp[:], ot[:], ident32_s[:FINAL, :FINAL])
                t = sb.tile([128, FINAL], F32, tag="t")
                nc.vector.tensor_add(t[:], tp[:], b3r_s[:])
                nmx = sb.tile([128, 1], F32, tag="nmx")
                nc.vector.reduce_max(nmx[:], t[:], axis=AX.X, negate=True)
                em = sb.tile([128, FINAL], F32, tag="em")
                sm = sb.tile([128, 1], F32, tag="sm")
                nc.scalar.activation(em[:], t[:], ACTF.Exp, bias=nmx[:, :1],
                                     accum_out=sm[:])
                ln = sb.tile([128, 1], F32, tag="ln")
                nc.scalar.activation(ln[:], sm[:], ACTF.Ln)
                o = sb.tile([128, FINAL], F32, tag="o")
                nc.vector.scalar_tensor_tensor(
                    out=o[:], in0=t[:], scalar=nmx[:, :1],
                    in1=ln[:].to_broadcast([128, FINAL]),
                    op0=ALU.add, op1=ALU.subtract)
                nc.sync.dma_start(out=outp[c * 128:(c + 1) * 128, :], in_=o[:])

    nc.compile()
    return nc


_CACHED = {}


def kernel(x, edge_index, Wl1, Wr1, att1, b1, Wl2, Wr2, att2, b2,
           Wl3, Wr3, att3, b3):
    per_core = _preprocess(x, edge_index)
    consts = _consts(Wl1, Wr1, att1, b1, Wl2, Wr2, att2, b2, Wl3, Wr3, att3, b3)

    if "nc" not in _CACHED:
        _CACHED["nc"] = _build()
    nc = _CACHED["nc"]

    in_maps = []
    for c in range(NCORE):
        m = dict(consts)
        m.update(per_core[c])
        in_maps.append(m)

    global LAST_EXEC_NS
    res = run_bass_kernel_spmd(nc, in_maps, core_ids=list(range(NCORE)),
                               trace=TRACE)
    LAST_EXEC_NS = res.exec_time_ns
    out = np.zeros((N, FINAL), dtype=np.float32)
    for c in range(NCORE):
        out[c * NLOC:(c + 1) * NLOC] = res.results[c]["outp"][:NLOC]
    return out
